# revision 16
# baseline (speedup 1.0000x reference)
"""BiDAF attention kernel for 8 Trainium2 NeuronCores (data-parallel over batch).

Contract: kernel(**inputs) takes the FULL unsharded inputs (as produced by the
reference setup_inputs) and returns the FULL [16, 1024, 2048] fp32 output.

Math (per batch b):
    s[i,j]  = c[i].c_w + q[j].q_w + sum_h c[i,h]*cqw[h]*q[j,h] + bias
    s1      = softmax_j(masked(s, q_mask));  s2 = softmax_i(masked(s, c_mask))
    a       = s1 @ q ; bb = s1 @ s2^T @ c
    out     = concat(c, a, c*a, c*bb)

Device mapping (per core: 2 batches):
  - Host folds cq_weight and c_weight into the q side:  qw'[j,h] = q*cqw + c_w
    so one PE matmul chain gives sT[j,i] = sim_cq[i,j] + sim_c[i].
  - sim_q + bias + q_mask fold into the Exp activation's per-partition bias.
  - c_mask (when non-trivial) is added via a K=1 rank-1 matmul.
  - Softmax without max-subtraction (values bounded, fp32-safe): one exp(sT)
    serves both softmaxes; normalizations are per-partition scales applied to
    the downstream matmul outputs.
  - t = s2T@c via PE-transposed exp(sT); a/b via e as stationary operand.
  - The c block of the output is assembled on the host (pure copy of an
    input); the device emits only the computed a | c*a | c*b blocks.

Precision modes (BIDAF_DTYPE): "mixed" (default; similarity chain fp32,
post-softmax matmuls float32r), "fp32" (all exact), "fp32r" (all reduced).
"""

import os
import sys
from contextlib import ExitStack

import numpy as np

for _p in ("/opt/trn_rl_repo", "/root/.axon_site/_ro/trn_rl_repo"):
    if os.path.isdir(_p) and _p not in sys.path:
        sys.path.append(_p)

B, CL, QL, H = 16, 1024, 128, 512
N_CORES = 8
BPC = B // N_CORES  # batches per core
NEG = np.float32(-1e30)

DTYPE_MODE = os.environ.get("BIDAF_DTYPE", "mixed")

_build_cache = {}


def _build(mask_trivial: bool, mode: str):
    key = (mask_trivial, mode)
    if key in _build_cache:
        return _build_cache[key]

    import concourse.bass as bass
    import concourse.tile as tile
    from concourse import bacc, mybir

    F32 = mybir.dt.float32
    F32R = mybir.dt.float32r
    SIM_DT = F32R if mode == "fp32r" else F32  # similarity-chain matmul dtype
    DOWN_DT = F32 if mode == "fp32" else F32R  # post-softmax matmul dtype
    AF = mybir.ActivationFunctionType
    PSUM = bass.MemorySpace.PSUM

    nc = bacc.Bacc("TRN2", target_bir_lowering=False, debug=False)

    c_d = nc.dram_tensor("c", [BPC, CL, H], F32, kind="ExternalInput")
    q_d = nc.dram_tensor("q", [BPC, QL, H], F32, kind="ExternalInput")
    qwT_d = nc.dram_tensor("qwT", [BPC, H, QL], F32, kind="ExternalInput")
    qbias_d = nc.dram_tensor("qbias", [BPC, QL, 1], F32, kind="ExternalInput")
    ident_d = nc.dram_tensor("ident", [128, 128], F32, kind="ExternalInput")
    if not mask_trivial:
        cmask_d = nc.dram_tensor("cmaskb", [BPC, 1, CL], F32, kind="ExternalInput")
        onesr_d = nc.dram_tensor("onesr", [1, QL], F32, kind="ExternalInput")
    onesc_d = nc.dram_tensor("onesc", [QL, 1], F32, kind="ExternalInput")
    out_d = nc.dram_tensor("out", [BPC, CL, 3 * H], F32, kind="ExternalOutput")

    KT = H // 128  # 4 k-tiles over the hidden dim
    IT = CL // 128  # 8 i-tiles over the context dim

    with tile.TileContext(nc) as tc, ExitStack() as ctx:
        const = ctx.enter_context(tc.tile_pool(name="const", bufs=1))
        sbp = ctx.enter_context(tc.tile_pool(name="sbp", bufs=2))
        outp = ctx.enter_context(tc.tile_pool(name="outp", bufs=3))
        ps_acc = ctx.enter_context(tc.tile_pool(name="ps_acc", bufs=2, space=PSUM))
        ps_tr = ctx.enter_context(tc.tile_pool(name="ps_tr", bufs=3, space=PSUM))
        ps_ab = ctx.enter_context(tc.tile_pool(name="ps_ab", bufs=2, space=PSUM))
        ps_cs = ctx.enter_context(tc.tile_pool(name="ps_cs", bufs=1, space=PSUM))

        # ---- PE clock warmup + ACT exp-table preload in the preamble window:
        # the HAM clock gate needs ~3.4us of sustained PE activity to lift the
        # PE from 1.2 to 2.4 GHz, and the first Exp pays a ~2.7us table load.
        # Both run on dummy data before the first input DMA lands.
        warm = const.tile([128, 2], F32, tag="warm")
        nc.gpsimd.memset(warm[:], 0.0)
        nc.scalar.activation(warm[:, 1:2], warm[:, 0:1], AF.Exp)
        pw = ps_cs.tile([128, IT], F32, tag="cs")
        for _ in range(56):
            nc.tensor.matmul(pw[:1, 0:1], warm[:, 0:1], warm[:, 1:2], start=True, stop=True)

        ident = const.tile([128, 128], F32, tag="ident")
        nc.sync.dma_start(ident[:], ident_d.ap())
        if SIM_DT != F32:
            ident_s = const.tile([128, 128], SIM_DT, tag="ident_s")
            nc.vector.tensor_copy(ident_s[:], ident[:])
        else:
            ident_s = ident
        if DOWN_DT != F32:
            ident_e = const.tile([128, 128], DOWN_DT, tag="ident_e")
            nc.vector.tensor_copy(ident_e[:], ident[:])
        else:
            ident_e = ident
        if not mask_trivial:
            cmask_f = const.tile([1, BPC * CL], F32, tag="cmask_f")
            nc.sync.dma_start(cmask_f[:], cmask_d.ap().rearrange("b one i -> one (b i)"))
            onesr_f = const.tile([1, QL], F32, tag="onesr_f")
            nc.sync.dma_start(onesr_f[:], onesr_d.ap())
            if SIM_DT != F32:
                cmask_all = const.tile([1, BPC * CL], SIM_DT, tag="cmask")
                nc.vector.tensor_copy(cmask_all[:], cmask_f[:])
                onesr = const.tile([1, QL], SIM_DT, tag="onesr")
                nc.vector.tensor_copy(onesr[:], onesr_f[:])
            else:
                cmask_all, onesr = cmask_f, onesr_f

        onesc_f = const.tile([QL, 1], F32, tag="onesc_f")
        nc.sync.dma_start(onesc_f[:], onesc_d.ap())

        # ---- phase A: emit ALL loads (both batches) on the SP HWDGE queue ----
        LD = []
        for bi in range(BPC):
            c_sb = []
            for it in range(IT):
                ct = sbp.tile([128, H], F32, tag=f"c{it}")
                nc.sync.dma_start(ct[:], c_d.ap()[bi, it * 128 : (it + 1) * 128, :])
                c_sb.append(ct)
            qwT_f = sbp.tile([128, KT, QL], F32, tag="qwT_f")
            nc.sync.dma_start(qwT_f[:], qwT_d.ap()[bi].rearrange("(t p) j -> p t j", p=128))
            qbias_sb = sbp.tile([QL, 1], F32, tag="qbias")
            nc.sync.dma_start(qbias_sb[:], qbias_d.ap()[bi])
            q_f = sbp.tile([QL, H], F32, tag="q_f")
            nc.sync.dma_start(q_f[:], q_d.ap()[bi])
            LD.append((c_sb, qwT_f, qbias_sb, q_f))

        # ---- phase B: per-batch compute + stores (stores also SP HWDGE) ----
        for bi in range(BPC):
            c_sb, qwT_f, qbias_sb, q_f = LD[bi]
            c_r = []
            for it in range(IT):
                if DOWN_DT != F32:
                    cr = sbp.tile([128, H], DOWN_DT, tag=f"cr{it}")
                    nc.vector.tensor_copy(cr[:], c_sb[it][:])
                    c_r.append(cr)
                else:
                    c_r.append(c_sb[it])
            if SIM_DT != F32:
                qwT_sb = sbp.tile([128, KT, QL], SIM_DT, tag="qwT_sb")
                nc.vector.tensor_copy(qwT_sb[:], qwT_f[:])
            else:
                qwT_sb = qwT_f
            if DOWN_DT != F32:
                q_sb = sbp.tile([QL, H], DOWN_DT, tag="q_sb")
                nc.vector.tensor_copy(q_sb[:], q_f[:])
            else:
                q_sb = q_f

            # ---- cT: transpose c per i-tile (4 h-blocks per PSUM tile) ----
            cta = sbp.tile([128, KT, 512], SIM_DT, tag="cta")  # cT[:, k, i], i in 0:512
            ctb = sbp.tile([128, KT, 512], SIM_DT, tag="ctb")
            csrc = c_r if SIM_DT != F32 else c_sb
            for it in range(IT):
                ptr = ps_tr.tile([128, 512], SIM_DT, tag="tr")
                for k in range(KT):
                    nc.tensor.transpose(
                        ptr[:, k * 128 : (k + 1) * 128],
                        csrc[it][:, k * 128 : (k + 1) * 128],
                        ident_s[:],
                    )
                dst = cta if it < 4 else ctb
                j = it % 4
                src = ptr[:].rearrange("p (k i) -> p k i", k=KT)
                if it % 2 == 0:
                    nc.vector.tensor_copy(dst[:, :, j * 128 : (j + 1) * 128], src)
                else:
                    nc.scalar.copy(dst[:, :, j * 128 : (j + 1) * 128], src)

            # ---- sT accumulation: psum[j, i] over two 512-wide halves ----
            sp = []
            for nh in range(2):
                spt = ps_acc.tile([QL, 512], F32, tag="acc")
                ct = cta if nh == 0 else ctb
                for k in range(KT):
                    nc.tensor.matmul(
                        spt[:],
                        qwT_sb[:, k, :],
                        ct[:, k, :],
                        start=(k == 0),
                        stop=(k == KT - 1 and mask_trivial),
                    )
                if not mask_trivial:
                    nc.tensor.matmul(
                        spt[:],
                        onesr[:],
                        cmask_all[:, bi * CL + nh * 512 : bi * CL + (nh + 1) * 512],
                        start=False,
                        stop=True,
                    )
                sp.append(spt)

            # ---- e = exp(sT + qbias), fused row-sums ----
            e_sb = sbp.tile([QL, CL], DOWN_DT, tag="e")
            rs2 = sbp.tile([QL, 2], F32, tag="rs2")
            for nh in range(2):
                nc.scalar.activation(
                    e_sb[:, nh * 512 : (nh + 1) * 512],
                    sp[nh][:],
                    AF.Exp,
                    bias=qbias_sb[:],
                    scale=1.0,
                    accum_out=rs2[:, nh : nh + 1],
                )
            rsum = sbp.tile([QL, 1], F32, tag="rsum")
            nc.vector.tensor_reduce(rsum[:], rs2[:], mybir.AxisListType.X, mybir.AluOpType.add)
            r2 = sbp.tile([QL, 1], F32, tag="r2")
            nc.vector.reciprocal(r2[:], rsum[:])

            # ---- column sums via tiny PE matmuls (e.T-slice @ ones) -> r1 ----
            pcs = ps_cs.tile([128, IT], F32, tag="cs")
            for it in range(IT):
                nc.tensor.matmul(
                    pcs[:, it : it + 1],
                    e_sb[:, it * 128 : (it + 1) * 128].bitcast(F32),
                    onesc_f[:],
                    start=True,
                    stop=True,
                )
            r1 = sbp.tile([128, IT], F32, tag="r1")
            nc.vector.reciprocal(r1[:], pcs[:])

            # ---- eN = e.T per half (feeds traw only) ----
            eN = []
            for half in range(2):
                pe = ps_tr.tile([128, 512], DOWN_DT, tag="tr")
                for j in range(4):
                    it = 4 * half + j
                    nc.tensor.transpose(
                        pe[:, j * 128 : (j + 1) * 128],
                        e_sb[:, it * 128 : (it + 1) * 128],
                        ident_e[:],
                    )
                eNh = sbp.tile([128, 4, 128], DOWN_DT, tag=f"eN{half}")
                if half == 0:
                    nc.vector.tensor_copy(eNh[:], pe[:].rearrange("p (t j) -> p t j", t=4))
                else:
                    nc.scalar.copy(eNh[:], pe[:].rearrange("p (t j) -> p t j", t=4))
                eN.append(eNh)

            # ---- traw = sum_i e[j,i] c[i,:]; t = r2 * traw ----
            ptraw = ps_acc.tile([QL, H], F32, tag="acc")
            for it in range(IT):
                nc.tensor.matmul(
                    ptraw[:],
                    eN[it // 4][:, it % 4, :],
                    c_r[it][:],
                    start=(it == 0),
                    stop=(it == IT - 1),
                )
            t_sb = sbp.tile([QL, H], DOWN_DT, tag="t")
            nc.scalar.mul(t_sb[:], ptraw[:], r2[:])

            # ---- per i-tile: a | c*a stored first, then c*b ----
            for it in range(IT):
                esl = e_sb[:, it * 128 : (it + 1) * 128]
                pa = ps_ab.tile([128, H], F32, tag="ab")
                nc.tensor.matmul(pa[:], esl, q_sb[:], start=True, stop=True)
                pb = ps_ab.tile([128, H], F32, tag="ab")
                nc.tensor.matmul(pb[:], esl, t_sb[:], start=True, stop=True)
                aca_sb = outp.tile([128, 2 * H], F32, tag="aca")
                b_sb = outp.tile([128, H], F32, tag="b")
                cb_sb = outp.tile([128, H], F32, tag="cb")
                nc.scalar.mul(aca_sb[:, 0:H], pa[:], r1[:, it : it + 1])
                nc.scalar.mul(b_sb[:], pb[:], r1[:, it : it + 1])
                nc.vector.tensor_mul(aca_sb[:, H : 2 * H], c_sb[it][:], aca_sb[:, 0:H])
                nc.vector.tensor_mul(cb_sb[:], c_sb[it][:], b_sb[:])
                rows = out_d.ap()[bi, it * 128 : (it + 1) * 128]
                nc.sync.dma_start(rows[:, 0 : 2 * H], aca_sb[:])
                nc.sync.dma_start(rows[:, 2 * H : 3 * H], cb_sb[:])

    nc.compile()
    _build_cache[key] = nc
    return nc


def _install_profshim():
    """Optional NTFF profiling support (BIDAF_PROFILE=1); self-contained."""
    import contextlib
    import ctypes
    import types

    if "antenv.axon_hooks" in sys.modules:
        return
    so_path = "/opt/axon/libaxon_pjrt.so"
    try:
        lib = ctypes.CDLL(so_path)
    except OSError:
        return
    if not hasattr(lib, "axon_start_nrt_profile"):
        return
    lib.axon_start_nrt_profile.argtypes = [ctypes.POINTER(ctypes.c_int64), ctypes.c_size_t]
    lib.axon_start_nrt_profile.restype = ctypes.c_int64
    lib.axon_stop_nrt_profile.argtypes = [ctypes.c_char_p]
    lib.axon_stop_nrt_profile.restype = ctypes.c_int64

    @contextlib.contextmanager
    def _hook(output_dir, device_ids):
        import jax

        jax.devices()
        if device_ids:
            ids = (ctypes.c_int64 * len(device_ids))(*device_ids)
            rc = lib.axon_start_nrt_profile(ids, len(device_ids))
        else:
            rc = lib.axon_start_nrt_profile(None, 0)
        if rc != 0:
            raise RuntimeError(f"axon_start_nrt_profile rc={rc}")
        try:
            yield
        finally:
            n = lib.axon_stop_nrt_profile(str(output_dir).encode())
            print(f"profile: {n} file(s) written to {output_dir}")

    mod = types.ModuleType("antenv.axon_hooks")
    mod.get_axon_ntff_profile_hook = lambda: _hook
    mod.set_axon_ntff_profile_hook = lambda h: None
    sys.modules["antenv.axon_hooks"] = mod
    import antenv

    antenv.axon_hooks = mod

    from concourse import bass_utils

    bass_utils.upload_artifacts = lambda tmpdir: f"local:{tmpdir}"


def kernel(c, q, c_mask, q_mask, c_weight, q_weight, cq_weight, bias):
    from concourse.bass_utils import run_bass_kernel_spmd

    c = np.asarray(c, dtype=np.float32)
    q = np.asarray(q, dtype=np.float32)
    c_mask = np.asarray(c_mask)
    q_mask = np.asarray(q_mask)
    c_weight = np.asarray(c_weight, dtype=np.float32)
    q_weight = np.asarray(q_weight, dtype=np.float32)
    cq_weight = np.asarray(cq_weight, dtype=np.float32)
    bias = np.asarray(bias, dtype=np.float32)

    # host-side folding (all tiny, O(B*(CL+QL)*H) at most)
    qw = q * cq_weight.reshape(1, 1, H) + c_weight.reshape(1, 1, H)  # [B, QL, H]
    qwT = np.ascontiguousarray(qw.transpose(0, 2, 1))  # [B, H, QL]
    sim_q = (q @ q_weight)[:, :, 0]  # [B, QL]
    amask_q = (1.0 - q_mask.astype(np.float32)) * NEG
    qbias = (sim_q + bias[0] + amask_q).reshape(B, QL, 1).astype(np.float32)
    amask_c = ((1.0 - c_mask.astype(np.float32)) * NEG).reshape(B, 1, CL)
    mask_trivial = bool((amask_c == 0).all())

    profile = os.environ.get("BIDAF_PROFILE", "") == "1"
    if profile:
        _install_profshim()

    nc = _build(mask_trivial, DTYPE_MODE)

    ident = np.eye(128, dtype=np.float32)
    onesr = np.ones((1, QL), dtype=np.float32)
    in_maps = []
    for core in range(N_CORES):
        s = slice(BPC * core, BPC * (core + 1))
        m = {
            "c": np.ascontiguousarray(c[s]),
            "q": np.ascontiguousarray(q[s]),
            "qwT": np.ascontiguousarray(qwT[s]),
            "qbias": np.ascontiguousarray(qbias[s]),
            "ident": ident,
            "onesc": np.ones((QL, 1), dtype=np.float32),
        }
        if not mask_trivial:
            m["cmaskb"] = np.ascontiguousarray(amask_c[s])
            m["onesr"] = onesr
        in_maps.append(m)

    kw = {}
    if profile:
        kw = dict(trace=True, tmpdir=os.environ.get("BIDAF_PROFILE_DIR") or None)
    res = run_bass_kernel_spmd(nc, in_maps, list(range(N_CORES)), **kw)
    if profile and res.exec_time_ns is not None:
        print(f"[kernel] HW exec time: {res.exec_time_ns} ns")
        kernel.last_exec_time_ns = res.exec_time_ns
        kernel.last_trace = res.instructions_and_trace[1] if res.instructions_and_trace else None

    out = np.empty((B, CL, 4 * H), dtype=np.float32)
    out[:, :, 0:H] = c
    for i in range(N_CORES):
        out[BPC * i : BPC * (i + 1), :, H:] = res.results[i]["out"]
    return out


kernel.last_exec_time_ns = None
kernel.last_trace = None
